# revision 10
# baseline (speedup 1.0000x reference)
"""FAST multi-head attention (p=2 Taylor linear attention) for Trainium2.

Self-contained: accepts FULL inputs q,k,v [2,16,4096,32] fp32, returns the
full output [2,16,4096,32]. Internally shards the 32 (b,h) pairs across 8
NeuronCores (4 per core) and runs one Bass/Tile kernel SPMD.

Math per (b,h)  (A0=1, A1=1, A2=0.5 Taylor coeffs of exp):
  num[n,e'] = sum_m v'[m,e'] * (A0 + A1 (q.k_m) + A2 (q.k_m)^2),  v' = [v | 1]
  out       = num[:, :32] / num[:, 32]
computed via a quadratic-feature (linear attention) factorization with a
cyclic pair cover + polarization so that:
  - k-side features (k_d1*k_d2 products) are built on the Vector engine,
  - q-side features are built as PE matmuls (q @ E) + ScalarE Square,
  - both contractions run on the PE at K=128/K=32 with row/col tiling.
"""
import dataclasses
import numpy as np

import concourse.bass as bass
import concourse.tile as tile
from concourse import mybir, bacc
from concourse.bass_utils import run_bass_kernel_spmd

F32 = mybir.dt.float32
A0, A1, A2 = 1.0, 1.0, 0.5
B, H, N, D = 2, 16, 4096, 32
NJ = 16                    # pair gaps j = 1..16 (cyclic cover)
F = NJ * D                 # 512 off-diagonal features
NCORES = 8
BH_PER_CORE = (B * H) // NCORES   # 4
NT = N // 128              # 32 n-tiles per (b,h)


def _host_consts():
    """E4 [128,512] (4-stacked sqrt-weighted pair selectors), HmT [128,4,32],
    ident [128,128]."""
    E = np.zeros((D, F), np.float32)
    Hm = np.zeros((D, F), np.float32)
    for jj in range(NJ):          # gap j = jj+1
        gap = jj + 1
        beta2 = A2 if gap < 16 else A2 / 2.0
        beta = np.sqrt(beta2).astype(np.float32)
        c = beta2 / A2
        for d1 in range(D):
            f = jj * D + d1
            d2 = (d1 + gap) % D
            E[d1, f] += beta
            E[d2, f] += beta
            Hm[d1, f] += c
            Hm[d2, f] += c
    E4 = np.tile(E, (4, 1)).astype(np.float32)               # [128, 512]
    HmT = Hm.T.reshape(4, 128, D).transpose(1, 0, 2).copy()  # [128, 4, 32]
    ident = np.eye(128, dtype=np.float32)
    return E4, HmT, ident


def _ap_free(x: bass.AP, free_ap, extra_offset=0):
    """AP with same partition dim as x but custom free-dim [step,count] list."""
    return dataclasses.replace(
        x, offset=x.offset + extra_offset, ap=[x.ap[0]] + [list(p) for p in free_ap]
    )


def build_nc(debug=False):
    nc = bacc.Bacc(None, target_bir_lowering=False)

    def tr(out_ap, in_ap, ident_ap, tile_position=None):
        nc.tensor.matmul(out_ap, in_ap, ident_ap, is_transpose=True,
                         tile_position=tile_position, skip_group_check=True)
    qin = nc.declare_dram_parameter("qin", [BH_PER_CORE, N, D], F32, isOutput=False)
    kin = nc.declare_dram_parameter("kin", [BH_PER_CORE, N, D], F32, isOutput=False)
    vin = nc.declare_dram_parameter("vin", [BH_PER_CORE, N, D], F32, isOutput=False)
    e4_in = nc.declare_dram_parameter("e4", [128, F], F32, isOutput=False)
    hmt_in = nc.declare_dram_parameter("hmt", [128, 4, D], F32, isOutput=False)
    id_in = nc.declare_dram_parameter("ident", [128, 128], F32, isOutput=False)
    out = nc.declare_dram_parameter("out", [BH_PER_CORE, N, D], F32, isOutput=True)
    dbg = {}
    if debug:
        for nm, shp in [("dbg_qtb", [128, 8, 128]), ("dbg_kvtc", [33, F]),
                        ("dbg_kvts", [33, 81]), ("dbg_phit", [128, 4, 512]),
                        ("dbg_wq4", [128, 33]), ("dbg_wd4", [128, 33]),
                        ("dbg_ansT", [33, 512]), ("dbg_khat0", [128, F])]:
            dbg[nm] = nc.declare_dram_parameter(nm, shp, F32, isOutput=True)

    SQ = mybir.ActivationFunctionType.Square
    sqrt_a2 = float(np.sqrt(A2))

    with tile.TileContext(nc) as tc:
        with (
            tc.tile_pool(name="sb_const", bufs=1) as sb_const,
            tc.tile_pool(name="sb_big", bufs=2) as sb_big,
            tc.tile_pool(name="sb_khat", bufs=3) as sb_khat,
            tc.tile_pool(name="sb_w", bufs=2) as sb_w,
            tc.tile_pool(name="sb_phi", bufs=2) as sb_phi,
            tc.tile_pool(name="sb_ep", bufs=3) as sb_ep,
            tc.tile_pool(name="ps_tr", bufs=2, space="PSUM") as ps_tr,
            tc.tile_pool(name="ps_kv", bufs=1, space="PSUM") as ps_kv,
            tc.tile_pool(name="ps_hk", bufs=1, space="PSUM") as ps_hk,
            tc.tile_pool(name="ps_u", bufs=1, space="PSUM") as ps_u,
            tc.tile_pool(name="ps_ans", bufs=1, space="PSUM") as ps_ans,
            tc.tile_pool(name="ps_at", bufs=1, space="PSUM") as ps_at,
        ):
            e4 = sb_const.tile([128, F], F32)
            nc.sync.dma_start(out=e4[:], in_=e4_in[:])
            hmt = sb_const.tile([128, 4, D], F32)
            nc.sync.dma_start(out=hmt[:], in_=hmt_in[:])
            ident = sb_const.tile([128, 128], F32)
            nc.sync.dma_start(out=ident[:], in_=id_in[:])

            for b in range(BH_PER_CORE):
                # q loaded permuted so each transpose block is contiguous:
                # q_sb[p, bb, a, d] = q[n = 128*(8a+bb)+p, d]
                qv = qin[b].rearrange("(a bb p) d -> p bb a d", a=4, bb=8)
                # (split per-a below: DMA AP balancing caps at 3 dims)
                kv_ = kin[b].rearrange("(t p) d -> p t d", p=128)
                vv = vin[b].rearrange("(t p) d -> p t d", p=128)
                ov = out[b].rearrange("(t p) d -> p t d", p=128)

                # ---------- phase 1: loads, k-features, MM-A, q transposes ----
                q_sb = sb_big.tile([128, 8, 4, D], F32, tag="q_sb")
                for a in range(4):
                    nc.sync.dma_start(out=q_sb[:, :, a, :], in_=qv[:, :, a, :])

                # staging [128, 32, 81]: [ones | k(32) | kwrap(16) | k2d(32)]
                stg = sb_big.tile([128, NT, 81], F32, tag="stg")
                nc.vector.memset(stg[:, :, 0:1], 1.0)
                nc.sync.dma_start(out=stg[:, :, 1:33], in_=kv_)
                nc.sync.dma_start(out=stg[:, :, 33:49], in_=kv_[:, :, 0:16])

                v_ext = sb_big.tile([128, NT, D + 1], F32, tag="v_ext")
                nc.vector.memset(v_ext[:, :, D:D + 1], 1.0)
                nc.sync.dma_start(out=v_ext[:, :, 0:D], in_=vv)

                kvt_s = ps_kv.tile([33, 81], F32, tag="kvt_s")   # [1|k|junk|k2d]^T v'
                kvt_c = ps_kv.tile([33, F], F32, tag="kvt_c")    # khat^T v'
                for t in range(NT):
                    # k2d = k*k
                    nc.vector.tensor_mul(
                        stg[:, t, 49:81], stg[:, t, 1:33], stg[:, t, 1:33]
                    )
                    # khat[p, jj*32+d] = k[p,d]*k[p,d+1+jj]
                    kbase = stg[:, t, 1:33]
                    in0 = _ap_free(kbase, [[0, NJ], [1, D]])
                    in1 = _ap_free(kbase, [[1, NJ], [1, D]], extra_offset=1)
                    khat = sb_khat.tile([128, F], F32, tag="khat")
                    nc.vector.tensor_mul(khat[:], in0, in1)
                    if debug and b == 0 and t == 0:
                        nc.sync.dma_start(out=dbg["dbg_khat0"][:], in_=khat[:])

                    lhs = v_ext[:, t, :]
                    nc.tensor.matmul(kvt_s[:], lhs, stg[:, t, :],
                                     start=(t == 0), stop=(t == NT - 1))
                    nc.tensor.matmul(kvt_c[:], lhs, khat[:],
                                     start=(t == 0), stop=(t == NT - 1))

                # q transposes: qTb[32a+d, bb, i] = q[n = 128*(8a+bb)+i, d]
                qtb = sb_big.tile([128, 8, 128], F32, tag="qtb")
                for bb in range(8):
                    src = q_sb[:, bb, :, :]
                    qt_ps = ps_tr.tile([128, 128], F32, tag="tr")
                    tr(qt_ps[:], src, ident[:])
                    nc.scalar.copy(out=qtb[:, bb, :], in_=qt_ps[:])

                if debug and b == 0:
                    nc.sync.dma_start(out=dbg["dbg_qtb"][:], in_=qtb[:])
                # PhiD^T = Square(sqrt(A2) * qT)
                phidt = sb_big.tile([128, 8, 128], F32, tag="phidt")
                nc.scalar.activation(out=phidt[:], in_=qtb[:], func=SQ,
                                     scale=sqrt_a2)

                # ---------- phase 2: weight assembly --------------------------
                kvt_s_sb = sb_w.tile([33, 81], F32, tag="kvt_s_sb")
                nc.vector.tensor_copy(kvt_s_sb[:], kvt_s[:])
                kvt_c_sb = sb_w.tile([33, F], F32, tag="kvt_c_sb")
                nc.vector.tensor_copy(kvt_c_sb[:], kvt_c[:])

                if debug and b == 0:
                    nc.sync.dma_start(out=dbg["dbg_kvts"][:], in_=kvt_s_sb[:])
                    nc.sync.dma_start(out=dbg["dbg_kvtc"][:], in_=kvt_c_sb[:])
                # wC slices: transpose kvt_c [33, 512] -> 4x [128, 33]
                wc = sb_w.tile([128, 4, 33], F32, tag="wc")
                for s in range(4):
                    trc = ps_tr.tile([128, 33], F32, tag="tr")
                    tr(trc[0:128, :], kvt_c_sb[:, 128 * s:128 * (s + 1)],
                       ident[0:33, 0:33])
                    nc.scalar.copy(out=wc[:, s, :], in_=trc[0:128, :])

                # HKVcT [33, 32] = sum_s wc_s^T(f,e')-contract with HmT chunks
                hk = ps_hk.tile([33, D], F32, tag="hk")
                for s in range(4):
                    nc.tensor.matmul(hk[:], wc[:, s, :], hmt[:, s, :],
                                     start=(s == 0), stop=(s == 3))

                # wDT [33, 32] = KV2ddT - HKVcT
                wdt = sb_w.tile([33, D], F32, tag="wdt")
                nc.vector.scalar_tensor_tensor(
                    out=wdt[:], in0=kvt_s_sb[:, 49:81], scalar=1.0, in1=hk[:],
                    op0=mybir.AluOpType.mult, op1=mybir.AluOpType.subtract,
                )

                # wQ4 / wD4: transpose to [32, 33] then replicate to 4
                # partition groups via small SBUF->SBUF DMAs
                wq4 = sb_w.tile([128, 33], F32, tag="wq4")
                wd4 = sb_w.tile([128, 33], F32, tag="wd4")
                trq = ps_tr.tile([128, 33], F32, tag="tr")
                trd = ps_tr.tile([128, 33], F32, tag="tr")
                tr(trq[0:32, :], kvt_s_sb[:, 1:33], ident[0:33, 0:33])
                tr(trd[0:32, :], wdt[:], ident[0:33, 0:33])
                nc.scalar.copy(out=wq4[0:32, :], in_=trq[0:32, :])
                nc.scalar.copy(out=wd4[0:32, :], in_=trd[0:32, :])
                for a in range(1, 4):
                    nc.sync.dma_start(out=wq4[32 * a:32 * a + 32, :],
                                      in_=wq4[0:32, :])
                    nc.sync.dma_start(out=wd4[32 * a:32 * a + 32, :],
                                      in_=wd4[0:32, :])

                if debug and b == 0:
                    nc.sync.dma_start(out=dbg["dbg_wq4"][:], in_=wq4[:])
                    nc.sync.dma_start(out=dbg["dbg_wd4"][:], in_=wd4[:])
                # ---------- phase 3: per 512-wide n-chunk ---------------------
                for c in range(8):
                    a, half = c // 2, c % 2
                    pa = slice(32 * a, 32 * a + 32)
                    blk = slice(4 * half, 4 * half + 4)

                    # U = E^T qT per f-slice; PhiOff^T = Square(U)
                    phit = sb_phi.tile([128, 4, 512], F32, tag="phit")
                    for s in range(4):
                        u_ps = ps_u.tile([128, 512], F32, tag="u")
                        nc.tensor.matmul(u_ps[:],
                                         e4[pa, 128 * s:128 * (s + 1)],
                                         qtb[pa, blk, :],
                                         tile_position=(32 * a, 0))
                        nc.scalar.activation(out=phit[:, s, :], in_=u_ps[:],
                                             func=SQ, scale=1.0)

                    if debug and b == 0 and c == 0:
                        nc.sync.dma_start(out=dbg["dbg_phit"][:], in_=phit[:])
                    # MM-B: ansT [33, 512]
                    ansT = ps_ans.tile([33, 512], F32, tag="ansT")
                    for s in range(4):
                        nc.tensor.matmul(ansT[:], wc[:, s, :], phit[:, s, :],
                                         start=(s == 0), stop=False)
                    nc.tensor.matmul(ansT[:], wd4[pa, :], phidt[pa, blk, :],
                                     start=False, stop=False,
                                     tile_position=(32 * a, 0))
                    nc.tensor.matmul(ansT[:], wq4[pa, :], qtb[pa, blk, :],
                                     start=False, stop=True,
                                     tile_position=(32 * a, 0))

                    # epilogue: + A0*sumv', transpose, divide, store
                    ansT_sb = sb_ep.tile([33, 512], F32, tag="ansT_sb")
                    nc.vector.tensor_scalar_add(ansT_sb[:], ansT[:],
                                                kvt_s_sb[:, 0:1])
                    if debug and b == 0 and c == 0:
                        nc.sync.dma_start(out=dbg["dbg_ansT"][:], in_=ansT_sb[:])
                    at = ps_at.tile([128, 4, 33], F32, tag="at")
                    for i in range(4):
                        tr(at[:, i, :], ansT_sb[:, 128 * i:128 * (i + 1)],
                           ident[0:33, 0:33])
                    r4 = sb_ep.tile([128, 4], F32, tag="r4")
                    nc.vector.reciprocal(r4[:], at[:, :, 32:33])
                    o_sb = sb_ep.tile([128, 4, D], F32, tag="o_sb")
                    nc.vector.tensor_mul(o_sb[:], at[:, :, 0:D],
                                         _ap_free(r4[:], [[1, 4], [0, D]]))
                    # rows of chunk c: tiles 8a + 4*half + i
                    t0 = 8 * a + 4 * half
                    nc.sync.dma_start(out=ov[:, t0:t0 + 4, :], in_=o_sb[:])

    nc.compile()
    return nc


_NC_CACHE = None


def _get_nc():
    global _NC_CACHE
    if _NC_CACHE is None:
        _NC_CACHE = build_nc()
    return _NC_CACHE


def _in_maps(q, k, v):
    qf = q.reshape(B * H, N, D)
    kf = k.reshape(B * H, N, D)
    vf = v.reshape(B * H, N, D)
    E4, HmT, ident = _host_consts()
    in_maps = []
    for c in range(NCORES):
        sl = slice(c * BH_PER_CORE, (c + 1) * BH_PER_CORE)
        in_maps.append({
            "qin": np.ascontiguousarray(qf[sl]),
            "kin": np.ascontiguousarray(kf[sl]),
            "vin": np.ascontiguousarray(vf[sl]),
            "e4": E4, "hmt": HmT, "ident": ident,
        })
    return in_maps


def run_traced(q, k, v):
    """Run once with trace=True to get HW exec time (test-only helper)."""
    q = np.ascontiguousarray(np.asarray(q, dtype=np.float32))
    k = np.ascontiguousarray(np.asarray(k, dtype=np.float32))
    v = np.ascontiguousarray(np.asarray(v, dtype=np.float32))
    nc = _get_nc()
    try:
        return run_bass_kernel_spmd(nc, _in_maps(q, k, v),
                                    core_ids=list(range(NCORES)), trace=True)
    except Exception as e:
        print("traced run failed:", e)
        return None


def kernel(q, k, v):
    q = np.ascontiguousarray(np.asarray(q, dtype=np.float32))
    k = np.ascontiguousarray(np.asarray(k, dtype=np.float32))
    v = np.ascontiguousarray(np.asarray(v, dtype=np.float32))
    assert q.shape == (B, H, N, D)

    nc = _get_nc()
    res = run_bass_kernel_spmd(nc, _in_maps(q, k, v),
                               core_ids=list(range(NCORES)))
    outs = [res.results[c]["out"] for c in range(NCORES)]
    full = np.concatenate(outs, axis=0).reshape(B, H, N, D)
    return full.astype(np.float32)


if __name__ == "__main__":
    rng = np.random.default_rng(0)
    q = rng.standard_normal((B, H, N, D), dtype=np.float32)
    k = rng.standard_normal((B, H, N, D), dtype=np.float32)
    v = rng.standard_normal((B, H, N, D), dtype=np.float32)
    o = kernel(q, k, v)
    print("ran", o.shape, o.dtype)
